# revision 20
# baseline (speedup 1.0000x reference)
"""Causal self-attention Trainium2 Bass kernel (software-pipelined).

Problem: B=128, T=256, D=512, H=8 heads of 64. f32 in/out.
Sharding: data-parallel over batch - 16 batches per NeuronCore, weights
replicated, no collectives.

Matmul datapath in fp16 (1 col/cycle moving-operand stream at 2.4GHz);
PSUM accumulation fp32. The N=512 projection matmuls are the dominant
irreducible PE cost (~110us/core), so the kernel keeps the PE dense and
warm end to end (matmul-queue idle measured <1us per run):

  1. Cross-pair software pipelining: attention of pair p (small MMs with
     ACT/GpSimd dependencies in the softmax chain) is interleaved at
     issue time with the N=512 projection groups of pair p+1 and the
     output projection of pair p-2, so the PE FIFO always has
     independent work and the HAM clock gate never re-throttles.
     Per-slot PE order: [T(s-2), S(s), G, O(s-1), G].
  2. Deferring the output projection by two pairs keeps the last pair's
     queue full (8 real groups); zero-weight filler matmuls cover the
     remaining tail slots to hold K=8/8.
  3. Weights are passed partition-major ([128, sec, k, n]) so startup
     DMAs land 4KB contiguous lines; loads are spread across the three
     DMA-capable engine queues (scalar=W_Q, sync=W_K + xt prefetch,
     gpsimd=xt0 + small tensors + W_V/W_out) and warm-up matmuls on a
     zeroed tile run inside the DGE-latency window so the first real
     projection starts at full clock.
  4. Attention math per (head-pair, batch) slot: S^T both s-tiles into
     one [128,384] PSUM bank (causality), single exp on ACT,
     multiplicative causal mask on GpSimd, O-matmuls carry a ones
     column whose output is the softmax denominator (per-partition),
     one reciprocal + broadcast-multiply on DVE normalizes into a
     head-pair staging tile, PE transposes feed the out-projection.
     Scale 1/sqrt(hd) and the V-path bias are folded on the host.
"""

import numpy as np

B, T, D = 128, 256, 512
H, HD = 8, 64
NCORES = 8
BL = B // NCORES  # batches per core


def build_nc(bl=BL, num_devices=NCORES):
    from contextlib import ExitStack

    import concourse.bacc as bacc
    import concourse.tile as tile
    from concourse import mybir

    f32 = mybir.dt.float32
    f16 = mybir.dt.float16
    AF = mybir.ActivationFunctionType

    nc = bacc.Bacc(
        "TRN2",
        target_bir_lowering=False,
        debug=False,
        enable_asserts=False,
        num_devices=num_devices,
    )

    npairs = bl // 2

    xt_d = nc.dram_tensor("xt", [bl, 128, 4, 256], f16, kind="ExternalInput").ap()
    w_d = nc.dram_tensor("wqkv", [128, 3, 4, 512], f16, kind="ExternalInput").ap()
    wo_d = nc.dram_tensor("wout", [128, 4, D], f16, kind="ExternalInput").ap()
    bqk_d = nc.dram_tensor("bqk", [128, 8], f32, kind="ExternalInput").ap()
    beff_d = nc.dram_tensor("beff", [128, D], f32, kind="ExternalInput").ap()
    bm_d = nc.dram_tensor("binm", [128, 128], f16, kind="ExternalInput").ap()
    id_d = nc.dram_tensor("ident", [128, 128], f16, kind="ExternalInput").ap()
    y_d = nc.dram_tensor("y", [bl, T, D], f32, kind="ExternalOutput").ap()

    with tile.TileContext(nc) as tc, ExitStack() as ctx:
        singles = ctx.enter_context(tc.tile_pool(name="singles", bufs=1))
        p_xt = ctx.enter_context(tc.tile_pool(name="p_xt", bufs=3))
        p_qkt = ctx.enter_context(tc.tile_pool(name="p_qkt", bufs=2))
        p_et = ctx.enter_context(tc.tile_pool(name="p_et", bufs=4))
        p_o = ctx.enter_context(tc.tile_pool(name="p_o", bufs=3))
        p_li = ctx.enter_context(tc.tile_pool(name="p_li", bufs=8))
        p_ot = ctx.enter_context(tc.tile_pool(name="p_ot", bufs=6))
        p_y = ctx.enter_context(tc.tile_pool(name="p_y", bufs=3))
        psA = ctx.enter_context(tc.tile_pool(name="psA", bufs=2, space="PSUM"))
        psB = ctx.enter_context(tc.tile_pool(name="psB", bufs=3, space="PSUM"))
        psC = ctx.enter_context(tc.tile_pool(name="psC", bufs=3, space="PSUM"))

        # ---- startup DMAs, spread across engine DGE queues ----
        w_sb = singles.tile([128, 3, 4, 512], f16, tag="w")
        nc.scalar.dma_start(out=w_sb[:, 0], in_=w_d[:, 0])  # Q (gates first MM)
        nc.sync.dma_start(out=w_sb[:, 1], in_=w_d[:, 1])  # K (gates f=4 group)
        # xt0 first on the gpsimd queue (gates first MM together with Q)
        xt0_tile = p_xt.tile([128, 2, 4, 256], f16, tag="xt", name="xt0t")
        for bb in range(2):
            nc.gpsimd.dma_start(out=xt0_tile[:, bb], in_=xt_d[bb])
        bqk_sb = singles.tile([128, 8], f32, tag="bqk")
        nc.gpsimd.dma_start(out=bqk_sb, in_=bqk_d)
        bm_sb = singles.tile([128, 128], f16, tag="bm")
        nc.gpsimd.dma_start(out=bm_sb, in_=bm_d)
        id_sb = singles.tile([128, 128], f16, tag="id")
        nc.gpsimd.dma_start(out=id_sb, in_=id_d)
        nc.gpsimd.dma_start(out=w_sb[:, 2], in_=w_d[:, 2])  # V
        wo_sb = singles.tile([128, 4, D], f16, tag="wo")
        nc.gpsimd.dma_start(out=wo_sb, in_=wo_d)
        beff_sb = singles.tile([128, D], f32, tag="beff")
        nc.gpsimd.dma_start(out=beff_sb, in_=beff_d)

        # persistent V tiles (pair double-buffer x batch): ones written once
        vas_db = []
        for i in range(2):
            row = []
            for j in range(2):
                va = singles.tile([128, 2, 8, 66], f16, tag=f"va{i}_{j}", name="va")
                nc.vector.memset(
                    va[:, :, :, 64:66].bitcast(mybir.dt.uint32), 0x3C003C00
                )
                row.append(va)
            vas_db.append(row)

        # zero tile for warm-up + the last pair's HAM-keepalive fillers
        zwu = singles.tile([128, 512], f16, tag="zwu")
        nc.vector.memset(zwu, 0.0)
        zps = psC.tile([128, 512], f32, tag="c", name="zps")
        for _ in range(16):
            nc.tensor.matmul(zps, lhsT=zwu[:, 0:128], rhs=zwu, start=True, stop=True)

        def load_xt(p, eng=None):
            t = p_xt.tile([128, 2, 4, 256], f16, tag="xt", name="xtt")
            e = eng if eng is not None else nc.sync
            for bb in range(2):
                e.dma_start(out=t[:, bb], in_=xt_d[p * 2 + bb])
            return t

        xts = {0: xt0_tile}
        if npairs > 1:
            xts[1] = load_xt(1)

        # ---- work-item helpers (each = one N=512 projection group) ----
        def qk_group(f, qkt, xt):
            qp = psC.tile([128, 2, 256], f32, tag="c")
            for k in range(4):
                nc.tensor.matmul(
                    qp,
                    lhsT=w_sb[:, f // 4, k, (f % 4) * 128 : (f % 4 + 1) * 128],
                    rhs=xt[:, :, k, :],
                    start=(k == 0),
                    stop=(k == 3),
                )
            if f % 2 == 0:
                nc.scalar.add(qkt[:, f], qp, bqk_sb[:, f : f + 1])
            else:
                nc.vector.tensor_scalar_add(qkt[:, f], qp, bqk_sb[:, f : f + 1])

        def v_group(bb, st, va, xt):
            vp = psC.tile([128, 512], f32, tag="c")
            for k in range(4):
                nc.tensor.matmul(
                    vp,
                    lhsT=xt[:, bb, k, st * 128 : (st + 1) * 128],
                    rhs=w_sb[:, 2, k, :],
                    start=(k == 0),
                    stop=(k == 3),
                )
            nc.scalar.activation(
                va[:, st, :, 0:64],
                vp.rearrange("p (h c) -> p h c", c=64),
                AF.Copy,
            )

        def outproj_group(p, bb, tt, otsb):
            yp = psC.tile([128, 512], f32, tag="c")
            for f in range(4):
                nc.tensor.matmul(
                    yp,
                    lhsT=otsb[:, f, tt * 128 : (tt + 1) * 128],
                    rhs=wo_sb[:, f, :],
                    start=(f == 0),
                    stop=(f == 3),
                )
            ysb = p_y.tile([128, 512], f32, tag="y")
            nc.vector.tensor_add(out=ysb, in0=yp, in1=beff_sb)
            qeng = nc.sync if (bb + tt) % 2 == 0 else nc.gpsimd
            qeng.dma_start(
                out=y_d[p * 2 + bb, tt * 128 : (tt + 1) * 128, :], in_=ysb
            )

        def run_item(item):
            kind = item[0]
            if kind == "qk":
                _, f, qkt, xt = item
                qk_group(f, qkt, xt)
            elif kind == "v":
                _, bb, st, va, xt = item
                v_group(bb, st, va, xt)
            else:
                _, p, bb, tt, otsb = item
                outproj_group(p, bb, tt, otsb)

        # ---- attention stage helpers ----
        def s_mms(s, qkt):
            fp, bb = s // 2, s % 2
            sps = []
            for st in range(2):
                for hh in range(2):
                    po = hh * 64
                    qt = qkt[po : po + 64, fp, bb, :]
                    kt = qkt[po : po + 64, 4 + fp, bb, :]
                    if st == 0:
                        sp = psB.tile([128, 384], f32, tag="s")
                        sps.append(sp)
                        nc.tensor.matmul(
                            sp[:, 0:256], lhsT=kt[:, 0:128], rhs=qt,
                            start=True, stop=True,
                        )
                    else:
                        nc.tensor.matmul(
                            sps[hh][:, 256:384], lhsT=kt[:, 128:256],
                            rhs=qt[:, 128:256], start=True, stop=True,
                        )
            return sps

        def exp_mask(sps):
            ets = []
            for hh in range(2):
                et = p_et.tile([128, 384], f16, tag="et")
                ets.append(et)
                nc.scalar.activation(et, sps[hh], AF.Exp)
                dv = et.rearrange("p (a c) -> p a c", a=3)[:, 0::2, :]
                nc.gpsimd.tensor_mul(
                    out=dv, in0=dv,
                    in1=bm_sb[:, None, :].broadcast_to([128, 2, 128]),
                )
            return ets

        def o_mms(s, ets, va):
            fp = s // 2
            osb = p_o.tile([128, 2, 128], f16, tag="o")
            for hh in range(2):
                h = 2 * fp + hh
                po = hh * 64
                et = ets[hh]
                op = psA.tile([128, 2, 66], f32, tag="a")
                nc.tensor.matmul(
                    op[:, 0, :], lhsT=et[:, 0:128], rhs=va[:, 0, h, :],
                    start=True, stop=True,
                )
                nc.tensor.matmul(
                    op[:, 1, :], lhsT=et[:, 128:256], rhs=va[:, 0, h, :],
                    start=True, stop=False,
                )
                nc.tensor.matmul(
                    op[:, 1, :], lhsT=et[:, 256:384], rhs=va[:, 1, h, :],
                    start=False, stop=True,
                )
                li = p_li.tile([128, 2], f32, tag="li")
                nc.vector.reciprocal(li, op[:, :, 64])
                nc.vector.tensor_mul(
                    out=osb[:, :, po : po + 64],
                    in0=op[:, :, 0:64],
                    in1=li[:, :, None].broadcast_to([128, 2, 64]),
                )
            return osb

        def t_mms(s, osb, otsb, eng_sel):
            fp = s // 2
            otp = psA.tile([128, 2, 128], f16, tag="a")
            for tt in range(2):
                nc.tensor.transpose(otp[:, tt, :], osb[:, tt, :], id_sb)
            nc.vector.tensor_copy(out=otsb[:, fp, :], in_=otp)

        # ---- prologue: QK(0) + V(0) ----
        qkt0 = p_qkt.tile([128, 8, 2, 256], f16, tag="qkt", name="qkt0")
        qkts = {0: qkt0}
        for f in (0, 1, 2, 3, 4, 5, 6, 7):
            qk_group(f, qkts[0], xts[0])
        for bb in range(2):
            for st in range(2):
                v_group(bb, st, vas_db[0][bb], xts[0])

        otsbs_by_pair = {}
        tpend = []  # transposes pending (depth-2 pipeline, crosses pairs)

        # ---- main loop ----
        for p in range(npairs):
            qkt = qkts.pop(p)
            va_pair = vas_db[p % 2]
            otsbs = [
                p_ot.tile([128, 4, 256], f16, tag="ot", name=f"ot{p}_{i}")
                for i in range(2)
            ]
            otsbs_by_pair[p] = otsbs

            # work queue of projection groups to interleave into this pair
            Q = []
            if p + 1 < npairs:
                qkts[p + 1] = p_qkt.tile([128, 8, 2, 256], f16, tag="qkt", name=f"qkt{p+1}")
                for f in range(8):
                    Q.append(("qk", f, qkts[p + 1], xts[p + 1]))
                for bb in range(2):
                    for st in range(2):
                        Q.append(("v", bb, st, vas_db[(p + 1) % 2][bb], xts[p + 1]))
                if p + 2 < npairs:
                    xts[p + 2] = load_xt(p + 2)
            for pp in ([p - 2] if p - 2 >= 0 else []):
                for bb in range(2):
                    for tt in range(2):
                        Q.append(("out", pp, bb, tt, otsbs_by_pair[pp][bb]))
            if p == npairs - 1 and p - 1 >= 0:
                for bb in range(2):
                    for tt in range(2):
                        Q.append(("out", p - 1, bb, tt, otsbs_by_pair[p - 1][bb]))
            if p - 3 in otsbs_by_pair:
                del otsbs_by_pair[p - 3]

            qi = 0  # queue cursor
            prev = None  # slot state awaiting O
            for s in range(8):
                # transpose from two slots back (osb guaranteed normalized)
                if tpend:
                    t_mms(*tpend.pop(0))
                sps = s_mms(s, qkt)
                ets = exp_mask(sps)

                # first projection group
                qtarget = (len(Q) * (2 * s + 1)) // 16
                while qi < qtarget:
                    run_item(Q[qi])
                    qi += 1
                # O for previous slot
                if prev is not None:
                    ps, pets = prev
                    osb = o_mms(ps, pets, va_pair[ps % 2])
                    tpend.append((ps, osb, otsbs[ps % 2], ps % 2 == 0))
                qtarget = (len(Q) * (2 * s + 2)) // 16
                while qi < qtarget:
                    run_item(Q[qi])
                    qi += 1
                if p == npairs - 1 and qi >= len(Q):
                    # keep the PE array busy so HAM stays at K=8/8
                    zp = psC.tile([128, 512], f32, tag="c", name="zp")
                    for _ in range(2):
                        nc.tensor.matmul(
                            zp, lhsT=zwu[:, 0:128], rhs=zwu, start=True, stop=True
                        )
                prev = (s, ets)

            # flush slot 7
            while qi < len(Q):
                run_item(Q[qi])
                qi += 1
            ps, pets = prev
            osb = o_mms(ps, pets, va_pair[ps % 2])
            tpend.append((ps, osb, otsbs[ps % 2], ps % 2 == 0))

        # ---- epilogue: drain transposes, out-projection of the last pair ----
        while tpend:
            t_mms(*tpend.pop(0))
        p = npairs - 1
        for bb in range(2):
            for tt in range(2):
                outproj_group(p, bb, tt, otsbs_by_pair[p][bb])

    nc.compile()
    return nc


def host_inputs(x, W_qkv, b_qkv, W_out, b_out):
    """Host-side preprocessing. Returns per-core-shared inputs plus the
    transposed x layout [B, 128, 4, 256] (d-major tiles)."""
    scale = 1.0 / np.sqrt(HD)
    W = np.array(W_qkv, dtype=np.float32).copy()
    W[:, :D] *= scale  # fold attention scale into Q projection
    bq = np.array(b_qkv, dtype=np.float64).copy()
    bq[:D] *= scale
    bqk = np.stack([bq[j * 128 : (j + 1) * 128] for j in range(8)], axis=1).astype(
        np.float32
    )
    beff_row = (
        np.array(b_qkv[2 * D :], np.float64) @ np.array(W_out, np.float64)
        + np.array(b_out, np.float64)
    ).astype(np.float32)
    beff = np.broadcast_to(beff_row, (128, D)).copy()
    i = np.arange(128)[:, None]
    j = np.arange(128)[None, :]
    binm = (j >= i).astype(np.float32)  # 1 on/above diagonal (t >= s)
    ident = np.eye(128, dtype=np.float32)
    # device weight layouts: partition-major so DMAs land contiguous lines
    # wqkv [128, 3, 4, 512]: [p, sec, k, n] = W[k*128+p, sec*512+n]
    wdev = (
        W.reshape(4, 128, 3, 512).transpose(1, 2, 0, 3).astype(np.float16)
    )
    wodev = (
        np.array(W_out, np.float32).reshape(4, 128, 512).transpose(1, 0, 2)
    ).astype(np.float16)
    return {
        "wqkv": np.ascontiguousarray(wdev),
        "wout": np.ascontiguousarray(wodev),
        "bqk": bqk,
        "beff": beff,
        "binm": binm.astype(np.float16),
        "ident": ident.astype(np.float16),
    }


def xt_layout(x):
    """[B, T, D] -> [B, 128, 4, 256]: xt[b, p, k, t] = x[b, t, 128k+p]."""
    xb = np.asarray(x, dtype=np.float32)
    return np.ascontiguousarray(
        xb.transpose(0, 2, 1).reshape(-1, 4, 128, T).transpose(0, 2, 1, 3)
    ).astype(np.float16)


def kernel(x, W_qkv, b_qkv, W_out, b_out):
    from concourse.bass_utils import run_bass_kernel_spmd

    shared = host_inputs(x, W_qkv, b_qkv, W_out, b_out)
    xt = xt_layout(x)
    nc = build_nc(BL, NCORES)
    in_maps = [
        {"xt": xt[c * BL : (c + 1) * BL], **shared} for c in range(NCORES)
    ]
    res = run_bass_kernel_spmd(nc, in_maps, core_ids=list(range(NCORES)))
    y = np.concatenate([res.results[c]["y"] for c in range(NCORES)], axis=0)
    return y.astype(np.float32)


# revision 26
# speedup vs baseline: 1.0316x; 1.0316x over previous
"""Causal self-attention Trainium2 Bass kernel (software-pipelined).

Problem: B=128, T=256, D=512, H=8 heads of 64. f32 in/out.
Sharding: data-parallel over batch - 16 batches per NeuronCore, weights
replicated, no collectives.

Matmul datapath in fp16 (1 col/cycle moving-operand stream at 2.4GHz);
PSUM accumulation fp32. The N=512 projection matmuls are the dominant
irreducible PE cost (~110us/core), so the kernel keeps the PE dense and
warm end to end (matmul-queue idle measured <1us per run):

  1. Cross-pair software pipelining: attention of pair p (small MMs with
     ACT/GpSimd dependencies in the softmax chain) is interleaved at
     issue time with the N=512 projection groups of pair p+1 and the
     output projection of pair p-2, so the PE FIFO always has
     independent work and the HAM clock gate never re-throttles.
     Per-slot PE order: [T(s-2), S(s), G, O(s-1), G].
  2. Deferring the output projection by two pairs keeps the last pair's
     queue full (8 real groups); zero-weight filler matmuls cover the
     remaining tail slots to hold K=8/8.
  3. Weights are passed partition-major ([128, sec, k, n]) so startup
     DMAs land 4KB contiguous lines; loads are spread across the three
     DMA-capable engine queues (scalar=W_Q, sync=W_K + xt prefetch,
     gpsimd=xt0 + small tensors + W_V/W_out) and warm-up matmuls on a
     zeroed tile run inside the DGE-latency window so the first real
     projection starts at full clock.
  4. Attention math per (head-pair, batch) slot: S^T both s-tiles into
     one [128,384] PSUM bank (causality), single exp on ACT,
     multiplicative causal mask on GpSimd, O-matmuls carry a ones
     column whose output is the softmax denominator (per-partition),
     one reciprocal + broadcast-multiply on DVE normalizes into a
     head-pair staging tile, PE transposes feed the out-projection.
     Scale 1/sqrt(hd) and the V-path bias are folded on the host.
"""

import numpy as np

B, T, D = 128, 256, 512
H, HD = 8, 64
NCORES = 8
BL = B // NCORES  # batches per core


def build_nc(bl=BL, num_devices=NCORES):
    from contextlib import ExitStack

    import concourse.bacc as bacc
    import concourse.tile as tile
    from concourse import mybir

    f32 = mybir.dt.float32
    f16 = mybir.dt.float16
    AF = mybir.ActivationFunctionType

    nc = bacc.Bacc(
        "TRN2",
        target_bir_lowering=False,
        debug=False,
        enable_asserts=False,
        num_devices=num_devices,
    )

    npairs = bl // 2

    xt_d = nc.dram_tensor("xt", [bl, 128, 4, 256], f16, kind="ExternalInput").ap()
    xt8_d = nc.dram_tensor("xt8", [bl, 128, 2, 2, 256], mybir.dt.float8e4, kind="ExternalInput").ap()
    w8_d = nc.dram_tensor("w8", [128, 2, 2, 2, 512], mybir.dt.float8e4, kind="ExternalInput").ap()
    wv_d = nc.dram_tensor("wv", [128, 4, 512], f16, kind="ExternalInput").ap()
    wo_d = nc.dram_tensor("wout", [128, 4, D], f16, kind="ExternalInput").ap()
    bqk_d = nc.dram_tensor("bqk", [128, 8], f32, kind="ExternalInput").ap()
    beff_d = nc.dram_tensor("beff", [128, D], f32, kind="ExternalInput").ap()
    bm_d = nc.dram_tensor("binm", [128, 128], f16, kind="ExternalInput").ap()
    id_d = nc.dram_tensor("ident", [128, 128], f16, kind="ExternalInput").ap()
    y_d = nc.dram_tensor("y", [bl, T, D], f32, kind="ExternalOutput").ap()

    with tile.TileContext(nc) as tc, ExitStack() as ctx:
        singles = ctx.enter_context(tc.tile_pool(name="singles", bufs=1))
        p_xt = ctx.enter_context(tc.tile_pool(name="p_xt", bufs=3))
        p_xt8 = ctx.enter_context(tc.tile_pool(name="p_xt8", bufs=3))
        p_qkt = ctx.enter_context(tc.tile_pool(name="p_qkt", bufs=3))
        p_et = ctx.enter_context(tc.tile_pool(name="p_et", bufs=6))
        p_o = ctx.enter_context(tc.tile_pool(name="p_o", bufs=4))
        p_li = ctx.enter_context(tc.tile_pool(name="p_li", bufs=8))
        p_ot = ctx.enter_context(tc.tile_pool(name="p_ot", bufs=6))
        p_y = ctx.enter_context(tc.tile_pool(name="p_y", bufs=4))
        psA = ctx.enter_context(tc.tile_pool(name="psA", bufs=2, space="PSUM"))
        psB = ctx.enter_context(tc.tile_pool(name="psB", bufs=4, space="PSUM"))
        psC = ctx.enter_context(tc.tile_pool(name="psC", bufs=2, space="PSUM"))

        # ---- startup DMAs, spread across engine DGE queues ----
        w8_sb = singles.tile([128, 2, 2, 2, 512], mybir.dt.float8e4, tag="w8")
        wv_sb = singles.tile([128, 4, 512], f16, tag="wv")
        nc.scalar.dma_start(out=w8_sb[:, 0], in_=w8_d[:, 0])  # Q8 (gates first MM)
        nc.sync.dma_start(out=w8_sb[:, 1], in_=w8_d[:, 1])  # K8 (gates f=4 group)
        # xt0/xt8(0) first on the gpsimd queue (gate first MMs together with Q8)
        xt80_tile = p_xt8.tile([128, 2, 2, 2, 256], mybir.dt.float8e4, tag="xt8", name="xt80t")
        for bb in range(2):
            nc.gpsimd.dma_start(out=xt80_tile[:, bb], in_=xt8_d[bb])
        xt0_tile = p_xt.tile([128, 2, 4, 256], f16, tag="xt", name="xt0t")
        for bb in range(2):
            nc.gpsimd.dma_start(out=xt0_tile[:, bb], in_=xt_d[bb])
        bqk_sb = singles.tile([128, 8], f32, tag="bqk")
        nc.gpsimd.dma_start(out=bqk_sb, in_=bqk_d)
        bm_sb = singles.tile([128, 128], f16, tag="bm")
        nc.gpsimd.dma_start(out=bm_sb, in_=bm_d)
        id_sb = singles.tile([128, 128], f16, tag="id")
        nc.gpsimd.dma_start(out=id_sb, in_=id_d)
        nc.gpsimd.dma_start(out=wv_sb, in_=wv_d)  # V
        wo_sb = singles.tile([128, 4, D], f16, tag="wo")
        nc.gpsimd.dma_start(out=wo_sb, in_=wo_d)
        beff_sb = singles.tile([128, D], f32, tag="beff")
        nc.gpsimd.dma_start(out=beff_sb, in_=beff_d)

        # persistent V tiles (pair double-buffer x batch): ones written once
        vas_db = []
        for i in range(2):
            row = []
            for j in range(2):
                va = singles.tile([128, 2, 8, 66], f16, tag=f"va{i}_{j}", name="va")
                nc.vector.memset(
                    va[:, :, :, 64:66].bitcast(mybir.dt.uint32), 0x3C003C00
                )
                row.append(va)
            vas_db.append(row)

        # zero tile for warm-up + the last pair's HAM-keepalive fillers
        zwu = singles.tile([128, 512], f16, tag="zwu")
        nc.vector.memset(zwu, 0.0)
        zps = psC.tile([128, 512], f32, tag="c", name="zps")
        for _ in range(16):
            nc.tensor.matmul(zps, lhsT=zwu[:, 0:128], rhs=zwu, start=True, stop=True)

        def load_xt(p, eng=None):
            t = p_xt.tile([128, 2, 4, 256], f16, tag="xt", name="xtt")
            t8 = p_xt8.tile([128, 2, 2, 2, 256], mybir.dt.float8e4, tag="xt8", name="xt8t")
            e = eng if eng is not None else nc.sync
            for bb in range(2):
                e.dma_start(out=t8[:, bb], in_=xt8_d[p * 2 + bb])
                e.dma_start(out=t[:, bb], in_=xt_d[p * 2 + bb])
            return (t, t8)

        xts = {0: (xt0_tile, xt80_tile)}
        if npairs > 1:
            xts[1] = load_xt(1)

        # ---- work-item helpers (each = one N=512 projection group) ----
        def qk_group(f, qkt, xtp):
            xt8 = xtp[1]
            qp = psC.tile([128, 2, 256], f32, tag="c")
            for bb in range(2):
                for kt in range(2):
                    nc.tensor.matmul(
                        qp[:, bb, :],
                        lhsT=w8_sb[:, f // 4, kt, :, (f % 4) * 128 : (f % 4 + 1) * 128],
                        rhs=xt8[:, bb, kt],
                        start=(kt == 0),
                        stop=(kt == 1),
                        perf_mode=mybir.MatmulPerfMode.DoubleRow,
                    )
            if f % 2 == 0:
                nc.scalar.add(qkt[:, f], qp, bqk_sb[:, f : f + 1])
            else:
                nc.vector.tensor_scalar_add(qkt[:, f], qp, bqk_sb[:, f : f + 1])

        def v_group(bb, st, va, xtp):
            xt = xtp[0]
            vp = psC.tile([128, 512], f32, tag="c")
            for k in range(4):
                nc.tensor.matmul(
                    vp,
                    lhsT=xt[:, bb, k, st * 128 : (st + 1) * 128],
                    rhs=wv_sb[:, k, :],
                    start=(k == 0),
                    stop=(k == 3),
                )
            nc.scalar.activation(
                va[:, st, :, 0:64],
                vp.rearrange("p (h c) -> p h c", c=64),
                AF.Copy,
            )

        def outproj_group(p, bb, tt, otsb):
            yp = psC.tile([128, 512], f32, tag="c")
            for f in range(4):
                nc.tensor.matmul(
                    yp,
                    lhsT=otsb[:, f, tt * 128 : (tt + 1) * 128],
                    rhs=wo_sb[:, f, :],
                    start=(f == 0),
                    stop=(f == 3),
                )
            ysb = p_y.tile([128, 512], f32, tag="y")
            nc.vector.tensor_add(out=ysb, in0=yp, in1=beff_sb)
            qeng = nc.sync if (bb + tt) % 2 == 0 else nc.gpsimd
            qeng.dma_start(
                out=y_d[p * 2 + bb, tt * 128 : (tt + 1) * 128, :], in_=ysb
            )

        def run_item(item):
            kind = item[0]
            if kind == "qk":
                _, f, qkt, xt = item
                qk_group(f, qkt, xt)
            elif kind == "v":
                _, bb, st, va, xt = item
                v_group(bb, st, va, xt)
            else:
                _, p, bb, tt, otsb = item
                outproj_group(p, bb, tt, otsb)

        # ---- attention stage helpers ----
        def s_mms(s, qkt):
            fp, bb = s // 2, s % 2
            sps = []
            for st in range(2):
                for hh in range(2):
                    po = hh * 64
                    qt = qkt[po : po + 64, fp, bb, :]
                    kt = qkt[po : po + 64, 4 + fp, bb, :]
                    if st == 0:
                        sp = psB.tile([128, 384], f32, tag="s")
                        sps.append(sp)
                        nc.tensor.matmul(
                            sp[:, 0:256], lhsT=kt[:, 0:128], rhs=qt,
                            start=True, stop=True,
                        )
                    else:
                        nc.tensor.matmul(
                            sps[hh][:, 256:384], lhsT=kt[:, 128:256],
                            rhs=qt[:, 128:256], start=True, stop=True,
                        )
            return sps

        def exp_mask(sps):
            ets = []
            for hh in range(2):
                et = p_et.tile([128, 384], f16, tag="et")
                ets.append(et)
                nc.scalar.activation(et, sps[hh], AF.Exp, scale=1.0 / 65536.0)
                dv = et.rearrange("p (a c) -> p a c", a=3)[:, 0::2, :]
                nc.gpsimd.tensor_mul(
                    out=dv, in0=dv,
                    in1=bm_sb[:, None, :].broadcast_to([128, 2, 128]),
                )
            return ets

        def o_mms(s, ets, va):
            fp = s // 2
            osb = p_o.tile([128, 2, 128], f16, tag="o")
            for hh in range(2):
                h = 2 * fp + hh
                po = hh * 64
                et = ets[hh]
                op = psA.tile([128, 2, 66], f32, tag="a")
                nc.tensor.matmul(
                    op[:, 0, :], lhsT=et[:, 0:128], rhs=va[:, 0, h, :],
                    start=True, stop=True,
                )
                nc.tensor.matmul(
                    op[:, 1, :], lhsT=et[:, 128:256], rhs=va[:, 0, h, :],
                    start=True, stop=False,
                )
                nc.tensor.matmul(
                    op[:, 1, :], lhsT=et[:, 256:384], rhs=va[:, 1, h, :],
                    start=False, stop=True,
                )
                li = p_li.tile([128, 2], f32, tag="li")
                nc.vector.reciprocal(li, op[:, :, 64])
                nc.vector.tensor_mul(
                    out=osb[:, :, po : po + 64],
                    in0=op[:, :, 0:64],
                    in1=li[:, :, None].broadcast_to([128, 2, 64]),
                )
            return osb

        def t_mms(s, osb, otsb, eng_sel):
            fp = s // 2
            otp = psB.tile([128, 2, 128], f16, tag="s")
            for tt in range(2):
                nc.tensor.transpose(otp[:, tt, :], osb[:, tt, :], id_sb)
            nc.vector.tensor_copy(out=otsb[:, fp, :], in_=otp)

        # ---- prologue: QK(0) + V(0) ----
        qkt0 = p_qkt.tile([128, 8, 2, 256], f16, tag="qkt", name="qkt0")
        qkts = {0: qkt0}
        for f in (0, 1, 2, 3, 4, 5, 6, 7):
            qk_group(f, qkts[0], xts[0])
        for bb in range(2):
            for st in range(2):
                v_group(bb, st, vas_db[0][bb], xts[0])

        otsbs_by_pair = {}
        tpend = []  # transposes pending (depth-2 pipeline, crosses pairs)

        # ---- main loop ----
        for p in range(npairs):
            qkt = qkts.pop(p)
            va_pair = vas_db[p % 2]
            otsbs = [
                p_ot.tile([128, 4, 256], f16, tag="ot", name=f"ot{p}_{i}")
                for i in range(2)
            ]
            otsbs_by_pair[p] = otsbs

            # work queue of projection groups to interleave into this pair
            Q = []
            if p + 1 < npairs:
                qkts[p + 1] = p_qkt.tile([128, 8, 2, 256], f16, tag="qkt", name=f"qkt{p+1}")
                for f in range(8):
                    Q.append(("qk", f, qkts[p + 1], xts[p + 1]))
                for bb in range(2):
                    for st in range(2):
                        Q.append(("v", bb, st, vas_db[(p + 1) % 2][bb], xts[p + 1]))
                if p + 2 < npairs:
                    xts[p + 2] = load_xt(p + 2)
            for pp in ([p - 2] if p - 2 >= 0 else []):
                for bb in range(2):
                    for tt in range(2):
                        Q.append(("out", pp, bb, tt, otsbs_by_pair[pp][bb]))
            if p == npairs - 1 and p - 1 >= 0:
                for bb in range(2):
                    for tt in range(2):
                        Q.append(("out", p - 1, bb, tt, otsbs_by_pair[p - 1][bb]))
            if p - 3 in otsbs_by_pair:
                del otsbs_by_pair[p - 3]

            qi = 0  # queue cursor
            prev = None  # slot state awaiting O
            for s in range(8):
                # transpose from two slots back (osb guaranteed normalized)
                if tpend:
                    t_mms(*tpend.pop(0))
                sps = s_mms(s, qkt)
                ets = exp_mask(sps)

                # first projection group
                qtarget = (len(Q) * (2 * s + 1)) // 16
                while qi < qtarget:
                    run_item(Q[qi])
                    qi += 1
                # O for previous slot
                if prev is not None:
                    ps, pets = prev
                    osb = o_mms(ps, pets, va_pair[ps % 2])
                    tpend.append((ps, osb, otsbs[ps % 2], ps % 2 == 0))
                qtarget = (len(Q) * (2 * s + 2)) // 16
                while qi < qtarget:
                    run_item(Q[qi])
                    qi += 1
                if p == npairs - 1 and qi >= len(Q):
                    # keep the PE array busy so HAM stays at K=8/8
                    zp = psC.tile([128, 512], f32, tag="c", name="zp")
                    for _ in range(2):
                        nc.tensor.matmul(
                            zp, lhsT=zwu[:, 0:128], rhs=zwu, start=True, stop=True
                        )
                prev = (s, ets)

            # flush slot 7
            while qi < len(Q):
                run_item(Q[qi])
                qi += 1
            ps, pets = prev
            osb = o_mms(ps, pets, va_pair[ps % 2])
            tpend.append((ps, osb, otsbs[ps % 2], ps % 2 == 0))

        # ---- epilogue: drain transposes, out-projection of the last pair ----
        while tpend:
            t_mms(*tpend.pop(0))
        p = npairs - 1
        for bb in range(2):
            for tt in range(2):
                outproj_group(p, bb, tt, otsbs_by_pair[p][bb])

    nc.compile()
    return nc


def host_inputs(x, W_qkv, b_qkv, W_out, b_out):
    """Host-side preprocessing. Returns per-core-shared inputs plus the
    transposed x layout [B, 128, 4, 256] (d-major tiles)."""
    scale = 1.0 / np.sqrt(HD)
    W = np.array(W_qkv, dtype=np.float32).copy()
    W[:, :D] *= scale  # fold attention scale into Q projection
    bq = np.array(b_qkv, dtype=np.float64).copy()
    bq[:D] *= scale
    bqk = np.stack([bq[j * 128 : (j + 1) * 128] for j in range(8)], axis=1).astype(
        np.float32
    )
    beff_row = (
        np.array(b_qkv[2 * D :], np.float64) @ np.array(W_out, np.float64)
        + np.array(b_out, np.float64)
    ).astype(np.float32)
    beff = np.broadcast_to(beff_row, (128, D)).copy()
    i = np.arange(128)[:, None]
    j = np.arange(128)[None, :]
    binm = (j >= i).astype(np.float32)  # 1 on/above diagonal (t >= s)
    ident = np.eye(128, dtype=np.float32)
    # device weight layouts: partition-major so DMAs land contiguous lines
    # wqkv [128, 3, 4, 512]: [p, sec, k, n] = W[k*128+p, sec*512+n]
    wdev = (
        W.reshape(4, 128, 3, 512).transpose(1, 2, 0, 3).astype(np.float16)
    )
    wodev = (
        np.array(W_out, np.float32).reshape(4, 128, 512).transpose(1, 0, 2)
    ).astype(np.float16)
    return {
        "wqkv": np.ascontiguousarray(wdev),
        "wout": np.ascontiguousarray(wodev),
        "bqk": bqk,
        "beff": beff,
        "binm": binm.astype(np.float16),
        "ident": ident.astype(np.float16),
    }


def xt_layout(x):
    """[B, T, D] -> [B, 128, 4, 256]: xt[b, p, k, t] = x[b, t, 128k+p]."""
    xb = np.asarray(x, dtype=np.float32)
    return np.ascontiguousarray(
        xb.transpose(0, 2, 1).reshape(-1, 4, 128, T).transpose(0, 2, 1, 3)
    ).astype(np.float16)


def kernel(x, W_qkv, b_qkv, W_out, b_out):
    from concourse.bass_utils import run_bass_kernel_spmd

    shared = host_inputs(x, W_qkv, b_qkv, W_out, b_out)
    xt = xt_layout(x)
    nc = build_nc(BL, NCORES)
    in_maps = [
        {"xt": xt[c * BL : (c + 1) * BL], **shared} for c in range(NCORES)
    ]
    res = run_bass_kernel_spmd(nc, in_maps, core_ids=list(range(NCORES)))
    y = np.concatenate([res.results[c]["y"] for c in range(NCORES)], axis=0)
    return y.astype(np.float32)


# revision 28
# speedup vs baseline: 1.0411x; 1.0092x over previous
"""Causal self-attention Trainium2 Bass kernel (software-pipelined).

Problem: B=128, T=256, D=512, H=8 heads of 64. f32 in/out.
Sharding: data-parallel over batch - 16 batches per NeuronCore, weights
replicated, no collectives.

Matmul datapath in fp16 (1 col/cycle moving-operand stream at 2.4GHz);
PSUM accumulation fp32. The N=512 projection matmuls are the dominant
irreducible PE cost (~110us/core), so the kernel keeps the PE dense and
warm end to end (matmul-queue idle measured <1us per run):

  1. Cross-pair software pipelining: attention of pair p (small MMs with
     ACT/GpSimd dependencies in the softmax chain) is interleaved at
     issue time with the N=512 projection groups of pair p+1 and the
     output projection of pair p-2, so the PE FIFO always has
     independent work and the HAM clock gate never re-throttles.
     Per-slot PE order: [T(s-2), S(s), G, O(s-1), G].
  2. Deferring the output projection by two pairs keeps the last pair's
     queue full (8 real groups); zero-weight filler matmuls cover the
     remaining tail slots to hold K=8/8.
  3. Weights are passed partition-major ([128, sec, k, n]) so startup
     DMAs land 4KB contiguous lines; loads are spread across the three
     DMA-capable engine queues (scalar=W_Q, sync=W_K + xt prefetch,
     gpsimd=xt0 + small tensors + W_V/W_out) and warm-up matmuls on a
     zeroed tile run inside the DGE-latency window so the first real
     projection starts at full clock.
  4. Attention math per (head-pair, batch) slot: S^T both s-tiles into
     one [128,384] PSUM bank (causality), single exp on ACT,
     multiplicative causal mask on GpSimd, O-matmuls carry a ones
     column whose output is the softmax denominator (per-partition),
     one reciprocal + broadcast-multiply on DVE normalizes into a
     head-pair staging tile, PE transposes feed the out-projection.
     Scale 1/sqrt(hd) and the V-path bias are folded on the host.
"""

import numpy as np

B, T, D = 128, 256, 512
H, HD = 8, 64
NCORES = 8
BL = B // NCORES  # batches per core


def build_nc(bl=BL, num_devices=NCORES):
    from contextlib import ExitStack

    import concourse.bacc as bacc
    import concourse.tile as tile
    from concourse import mybir

    f32 = mybir.dt.float32
    f16 = mybir.dt.float16
    AF = mybir.ActivationFunctionType

    nc = bacc.Bacc(
        "TRN2",
        target_bir_lowering=False,
        debug=False,
        enable_asserts=False,
        num_devices=num_devices,
    )

    npairs = bl // 2

    xt_d = nc.dram_tensor("xt", [bl, 128, 4, 256], f16, kind="ExternalInput").ap()
    xt8_d = nc.dram_tensor("xt8", [bl, 128, 2, 2, 256], mybir.dt.float8e4, kind="ExternalInput").ap()
    w8_d = nc.dram_tensor("w8", [128, 2, 2, 2, 512], mybir.dt.float8e4, kind="ExternalInput").ap()
    wv_d = nc.dram_tensor("wv", [128, 4, 512], f16, kind="ExternalInput").ap()
    wo_d = nc.dram_tensor("wout", [128, 4, D], f16, kind="ExternalInput").ap()
    bqk_d = nc.dram_tensor("bqk", [128, 8], f32, kind="ExternalInput").ap()
    beff_d = nc.dram_tensor("beff", [128, D], f32, kind="ExternalInput").ap()
    bm_d = nc.dram_tensor("binm", [128, 128], f16, kind="ExternalInput").ap()
    id_d = nc.dram_tensor("ident", [128, 128], f16, kind="ExternalInput").ap()
    y_d = nc.dram_tensor("y", [bl, T, D], f32, kind="ExternalOutput").ap()

    with tile.TileContext(nc) as tc, ExitStack() as ctx:
        singles = ctx.enter_context(tc.tile_pool(name="singles", bufs=1))
        p_xt = ctx.enter_context(tc.tile_pool(name="p_xt", bufs=3))
        p_xt8 = ctx.enter_context(tc.tile_pool(name="p_xt8", bufs=3))
        p_qkt = ctx.enter_context(tc.tile_pool(name="p_qkt", bufs=3))
        p_et = ctx.enter_context(tc.tile_pool(name="p_et", bufs=6))
        p_o = ctx.enter_context(tc.tile_pool(name="p_o", bufs=4))
        p_li = ctx.enter_context(tc.tile_pool(name="p_li", bufs=8))
        p_ot = ctx.enter_context(tc.tile_pool(name="p_ot", bufs=6))
        p_y = ctx.enter_context(tc.tile_pool(name="p_y", bufs=4))
        psA = ctx.enter_context(tc.tile_pool(name="psA", bufs=2, space="PSUM"))
        psB = ctx.enter_context(tc.tile_pool(name="psB", bufs=3, space="PSUM"))
        psC = ctx.enter_context(tc.tile_pool(name="psC", bufs=3, space="PSUM"))

        # ---- startup DMAs, spread across engine DGE queues ----
        w8_sb = singles.tile([128, 2, 2, 2, 512], mybir.dt.float8e4, tag="w8")
        wv_sb = singles.tile([128, 4, 512], f16, tag="wv")
        nc.scalar.dma_start(out=w8_sb[:, 0], in_=w8_d[:, 0])  # Q8 (gates first MM)
        nc.sync.dma_start(out=w8_sb[:, 1], in_=w8_d[:, 1])  # K8 (gates f=4 group)
        # xt0/xt8(0) first on the gpsimd queue (gate first MMs together with Q8)
        xt80_tile = p_xt8.tile([128, 2, 2, 2, 256], mybir.dt.float8e4, tag="xt8", name="xt80t")
        for bb in range(2):
            nc.gpsimd.dma_start(out=xt80_tile[:, bb], in_=xt8_d[bb])
        xt0_tile = p_xt.tile([128, 2, 4, 256], f16, tag="xt", name="xt0t")
        for bb in range(2):
            nc.gpsimd.dma_start(out=xt0_tile[:, bb], in_=xt_d[bb])
        bqk_sb = singles.tile([128, 8], f32, tag="bqk")
        nc.gpsimd.dma_start(out=bqk_sb, in_=bqk_d)
        bm_sb = singles.tile([128, 128], f16, tag="bm")
        nc.gpsimd.dma_start(out=bm_sb, in_=bm_d)
        id_sb = singles.tile([128, 128], f16, tag="id")
        nc.gpsimd.dma_start(out=id_sb, in_=id_d)
        nc.gpsimd.dma_start(out=wv_sb, in_=wv_d)  # V
        wo_sb = singles.tile([128, 4, D], f16, tag="wo")
        nc.gpsimd.dma_start(out=wo_sb, in_=wo_d)
        beff_sb = singles.tile([128, D], f32, tag="beff")
        nc.gpsimd.dma_start(out=beff_sb, in_=beff_d)

        # persistent V tiles (pair double-buffer x batch): ones written once
        vas_db = []
        for i in range(2):
            row = []
            for j in range(2):
                va = singles.tile([128, 2, 8, 66], f16, tag=f"va{i}_{j}", name="va")
                nc.vector.memset(
                    va[:, :, :, 64:66].bitcast(mybir.dt.uint32), 0x3C003C00
                )
                row.append(va)
            vas_db.append(row)

        # zero tile for warm-up + the last pair's HAM-keepalive fillers
        zwu = singles.tile([128, 512], f16, tag="zwu")
        nc.vector.memset(zwu, 0.0)
        zps = psC.tile([128, 512], f32, tag="c", name="zps")
        for _ in range(10):
            nc.tensor.matmul(zps, lhsT=zwu[:, 0:128], rhs=zwu, start=True, stop=True)

        def load_xt(p, eng=None):
            t = p_xt.tile([128, 2, 4, 256], f16, tag="xt", name="xtt")
            t8 = p_xt8.tile([128, 2, 2, 2, 256], mybir.dt.float8e4, tag="xt8", name="xt8t")
            e = eng if eng is not None else nc.sync
            for bb in range(2):
                e.dma_start(out=t8[:, bb], in_=xt8_d[p * 2 + bb])
                e.dma_start(out=t[:, bb], in_=xt_d[p * 2 + bb])
            return (t, t8)

        xts = {0: (xt0_tile, xt80_tile)}
        if npairs > 1:
            xts[1] = load_xt(1)

        # ---- work-item helpers (each = one N=512 projection group) ----
        def qk_group(f, qkt, xtp):
            xt8 = xtp[1]
            qp = psC.tile([128, 2, 256], f32, tag="c")
            for bb in range(2):
                for kt in range(2):
                    nc.tensor.matmul(
                        qp[:, bb, :],
                        lhsT=w8_sb[:, f // 4, kt, :, (f % 4) * 128 : (f % 4 + 1) * 128],
                        rhs=xt8[:, bb, kt],
                        start=(kt == 0),
                        stop=(kt == 1),
                        perf_mode=mybir.MatmulPerfMode.DoubleRow,
                    )
            if f % 2 == 0:
                nc.scalar.add(qkt[:, f], qp, bqk_sb[:, f : f + 1])
            else:
                nc.vector.tensor_scalar_add(qkt[:, f], qp, bqk_sb[:, f : f + 1])

        def v_group(bb, st, va, xtp):
            xt = xtp[0]
            vp = psC.tile([128, 512], f32, tag="c")
            for k in range(4):
                nc.tensor.matmul(
                    vp,
                    lhsT=xt[:, bb, k, st * 128 : (st + 1) * 128],
                    rhs=wv_sb[:, k, :],
                    start=(k == 0),
                    stop=(k == 3),
                )
            nc.scalar.activation(
                va[:, st, :, 0:64],
                vp.rearrange("p (h c) -> p h c", c=64),
                AF.Copy,
            )

        def outproj_group(p, bb, tt, otsb):
            yp = psC.tile([128, 512], f32, tag="c")
            for f in range(4):
                nc.tensor.matmul(
                    yp,
                    lhsT=otsb[:, f, tt * 128 : (tt + 1) * 128],
                    rhs=wo_sb[:, f, :],
                    start=(f == 0),
                    stop=(f == 3),
                )
            ysb = p_y.tile([128, 512], f32, tag="y")
            nc.vector.tensor_add(out=ysb, in0=yp, in1=beff_sb)
            qeng = nc.sync if (bb + tt) % 2 == 0 else nc.gpsimd
            qeng.dma_start(
                out=y_d[p * 2 + bb, tt * 128 : (tt + 1) * 128, :], in_=ysb
            )

        def run_item(item):
            kind = item[0]
            if kind == "qk":
                _, f, qkt, xt = item
                qk_group(f, qkt, xt)
            elif kind == "v":
                _, bb, st, va, xt = item
                v_group(bb, st, va, xt)
            else:
                _, p, bb, tt, otsb = item
                outproj_group(p, bb, tt, otsb)

        # ---- attention stage helpers ----
        def s_mms(s, qkt):
            fp, bb = s // 2, s % 2
            sps = []
            for st in range(2):
                for hh in range(2):
                    po = hh * 64
                    qt = qkt[po : po + 64, fp, bb, :]
                    kt = qkt[po : po + 64, 4 + fp, bb, :]
                    if st == 0:
                        sp = psB.tile([128, 384], f32, tag="s")
                        sps.append(sp)
                        nc.tensor.matmul(
                            sp[:, 0:256], lhsT=kt[:, 0:128], rhs=qt,
                            start=True, stop=True,
                        )
                    else:
                        nc.tensor.matmul(
                            sps[hh][:, 256:384], lhsT=kt[:, 128:256],
                            rhs=qt[:, 128:256], start=True, stop=True,
                        )
            return sps

        def exp_mask(sps):
            ets = []
            for hh in range(2):
                et = p_et.tile([128, 384], f16, tag="et")
                ets.append(et)
                nc.scalar.activation(et, sps[hh], AF.Exp, scale=1.0 / 65536.0)
                dv = et.rearrange("p (a c) -> p a c", a=3)[:, 0::2, :]
                nc.gpsimd.tensor_mul(
                    out=dv, in0=dv,
                    in1=bm_sb[:, None, :].broadcast_to([128, 2, 128]),
                )
            return ets

        def o_mms(s, ets, va):
            fp = s // 2
            osb = p_o.tile([128, 2, 128], f16, tag="o")
            for hh in range(2):
                h = 2 * fp + hh
                po = hh * 64
                et = ets[hh]
                op = psA.tile([128, 2, 66], f32, tag="a")
                nc.tensor.matmul(
                    op[:, 0, :], lhsT=et[:, 0:128], rhs=va[:, 0, h, :],
                    start=True, stop=True,
                )
                nc.tensor.matmul(
                    op[:, 1, :], lhsT=et[:, 128:256], rhs=va[:, 0, h, :],
                    start=True, stop=False,
                )
                nc.tensor.matmul(
                    op[:, 1, :], lhsT=et[:, 256:384], rhs=va[:, 1, h, :],
                    start=False, stop=True,
                )
                li = p_li.tile([128, 2], f32, tag="li")
                nc.vector.reciprocal(li, op[:, :, 64])
                nc.vector.tensor_mul(
                    out=osb[:, :, po : po + 64],
                    in0=op[:, :, 0:64],
                    in1=li[:, :, None].broadcast_to([128, 2, 64]),
                )
            return osb

        def t_mms(s, osb, otsb, eng_sel):
            fp = s // 2
            otp = psB.tile([128, 2, 128], f16, tag="s")
            for tt in range(2):
                nc.tensor.transpose(otp[:, tt, :], osb[:, tt, :], id_sb)
            nc.vector.tensor_copy(out=otsb[:, fp, :], in_=otp)

        # ---- prologue: QK(0) + V(0) ----
        qkt0 = p_qkt.tile([128, 8, 2, 256], f16, tag="qkt", name="qkt0")
        qkts = {0: qkt0}
        for f in (0, 1, 2, 3, 4, 5, 6, 7):
            qk_group(f, qkts[0], xts[0])
        for bb in range(2):
            for st in range(2):
                v_group(bb, st, vas_db[0][bb], xts[0])

        otsbs_by_pair = {}
        tpend = []  # transposes pending (depth-2 pipeline, crosses pairs)

        # ---- main loop ----
        for p in range(npairs):
            qkt = qkts.pop(p)
            va_pair = vas_db[p % 2]
            otsbs = [
                p_ot.tile([128, 4, 256], f16, tag="ot", name=f"ot{p}_{i}")
                for i in range(2)
            ]
            otsbs_by_pair[p] = otsbs

            # work queue of projection groups to interleave into this pair
            Q = []
            if p + 1 < npairs:
                qkts[p + 1] = p_qkt.tile([128, 8, 2, 256], f16, tag="qkt", name=f"qkt{p+1}")
                for f in range(8):
                    Q.append(("qk", f, qkts[p + 1], xts[p + 1]))
                for bb in range(2):
                    for st in range(2):
                        Q.append(("v", bb, st, vas_db[(p + 1) % 2][bb], xts[p + 1]))
                if p + 2 < npairs:
                    xts[p + 2] = load_xt(p + 2)
            for pp in ([p - 2] if p - 2 >= 0 else []):
                for bb in range(2):
                    for tt in range(2):
                        Q.append(("out", pp, bb, tt, otsbs_by_pair[pp][bb]))
            if p == npairs - 1 and p - 1 >= 0:
                for bb in range(2):
                    for tt in range(2):
                        Q.append(("out", p - 1, bb, tt, otsbs_by_pair[p - 1][bb]))
            if p - 3 in otsbs_by_pair:
                del otsbs_by_pair[p - 3]

            qi = 0  # queue cursor
            prev = None  # slot state awaiting O
            for s in range(8):
                # transpose from two slots back (osb guaranteed normalized)
                if tpend:
                    t_mms(*tpend.pop(0))
                sps = s_mms(s, qkt)
                ets = exp_mask(sps)

                # first projection group
                qtarget = (len(Q) * (2 * s + 1)) // 16
                while qi < qtarget:
                    run_item(Q[qi])
                    qi += 1
                # O for previous slot
                if prev is not None:
                    ps, pets = prev
                    osb = o_mms(ps, pets, va_pair[ps % 2])
                    tpend.append((ps, osb, otsbs[ps % 2], ps % 2 == 0))
                qtarget = (len(Q) * (2 * s + 2)) // 16
                while qi < qtarget:
                    run_item(Q[qi])
                    qi += 1
                if p == npairs - 1 and qi >= len(Q):
                    # keep the PE array busy so HAM stays at K=8/8
                    zp = psC.tile([128, 512], f32, tag="c", name="zp")
                    for _ in range(2):
                        nc.tensor.matmul(
                            zp, lhsT=zwu[:, 0:128], rhs=zwu, start=True, stop=True
                        )
                prev = (s, ets)

            # flush slot 7
            while qi < len(Q):
                run_item(Q[qi])
                qi += 1
            ps, pets = prev
            osb = o_mms(ps, pets, va_pair[ps % 2])
            tpend.append((ps, osb, otsbs[ps % 2], ps % 2 == 0))

        # ---- epilogue: drain transposes, out-projection of the last pair ----
        while tpend:
            t_mms(*tpend.pop(0))
        p = npairs - 1
        for bb in range(2):
            for tt in range(2):
                outproj_group(p, bb, tt, otsbs_by_pair[p][bb])

    nc.compile()
    return nc


def host_inputs(x, W_qkv, b_qkv, W_out, b_out):
    """Host-side preprocessing. Returns per-core-shared inputs plus the
    transposed x layout [B, 128, 4, 256] (d-major tiles)."""
    scale = 1.0 / np.sqrt(HD)
    W = np.array(W_qkv, dtype=np.float32).copy()
    W[:, :D] *= scale  # fold attention scale into Q projection
    bq = np.array(b_qkv, dtype=np.float64).copy()
    bq[:D] *= scale
    bqk = np.stack([bq[j * 128 : (j + 1) * 128] for j in range(8)], axis=1).astype(
        np.float32
    )
    beff_row = (
        np.array(b_qkv[2 * D :], np.float64) @ np.array(W_out, np.float64)
        + np.array(b_out, np.float64)
    ).astype(np.float32)
    beff = np.broadcast_to(beff_row, (128, D)).copy()
    i = np.arange(128)[:, None]
    j = np.arange(128)[None, :]
    binm = (j >= i).astype(np.float32)  # 1 on/above diagonal (t >= s)
    ident = np.eye(128, dtype=np.float32)
    # device weight layouts: partition-major so DMAs land contiguous lines
    # wqkv [128, 3, 4, 512]: [p, sec, k, n] = W[k*128+p, sec*512+n]
    wdev = (
        W.reshape(4, 128, 3, 512).transpose(1, 2, 0, 3).astype(np.float16)
    )
    wodev = (
        np.array(W_out, np.float32).reshape(4, 128, 512).transpose(1, 0, 2)
    ).astype(np.float16)
    return {
        "wqkv": np.ascontiguousarray(wdev),
        "wout": np.ascontiguousarray(wodev),
        "bqk": bqk,
        "beff": beff,
        "binm": binm.astype(np.float16),
        "ident": ident.astype(np.float16),
    }


def xt_layout(x):
    """[B, T, D] -> [B, 128, 4, 256]: xt[b, p, k, t] = x[b, t, 128k+p]."""
    xb = np.asarray(x, dtype=np.float32)
    return np.ascontiguousarray(
        xb.transpose(0, 2, 1).reshape(-1, 4, 128, T).transpose(0, 2, 1, 3)
    ).astype(np.float16)


def kernel(x, W_qkv, b_qkv, W_out, b_out):
    from concourse.bass_utils import run_bass_kernel_spmd

    shared = host_inputs(x, W_qkv, b_qkv, W_out, b_out)
    xt = xt_layout(x)
    nc = build_nc(BL, NCORES)
    in_maps = [
        {"xt": xt[c * BL : (c + 1) * BL], **shared} for c in range(NCORES)
    ]
    res = run_bass_kernel_spmd(nc, in_maps, core_ids=list(range(NCORES)))
    y = np.concatenate([res.results[c]["y"] for c in range(NCORES)], axis=0)
    return y.astype(np.float32)
